# revision 1
# baseline (speedup 1.0000x reference)
"""Trainium2 Bass kernel for causal self-attention (B=4, T=2048, C=2048, H=16).

Sharding: 8 cores = 4 batches x 2 head-groups (8 heads each).
All-fp16 datapath (PSUM accumulation fp32). Per core:
  A) v = x @ Wv            -> fp16 spill [T, 1024]
  B+C merged, per head h:
    B-block: qkT features (q_h, k_h) = Wqk^T x^T + RoPE -> resident fp16
    C-block: flash-style SDPA, t-outer/j-inner, software-pipelined:
      scores mm -> exp (scalar) -> [diag tri-mul] -> p_sum += p (vector)
      -> PV mm accumulates numerator in PSUM (tensor, lagged 2 items)
      per t: ones-mm denominator from p_sum -> recip_approx (vector)
      -> y = psy * rden resident fp16
  D) partial_out = y^T @ wp -> [T, C] fp16 partial
Host sums core pairs per batch, adds b_proj and the folded bias row
bv @ wp (token-independent).
"""

import sys

import numpy as np

sys.path.insert(0, "/opt/trn_rl_repo")

import concourse.bass as bass  # noqa: E402,F401
import concourse.mybir as mybir  # noqa: E402
import concourse.tile as tile  # noqa: E402
from concourse import bacc  # noqa: E402

F32 = mybir.dt.float32
F16 = mybir.dt.float16
AF = mybir.ActivationFunctionType

B, T, C = 4, 2048, 2048
H, D = 16, 128
HPC = 8            # heads per core
P = 128
NT = 512           # matmul moving free dim
TT = T // NT       # 4 token tiles
CC = C // P        # 16 contraction chunks over C
NF = 2 * HPC       # 16 feature chunks, interleaved (q_h, k_h) per head
ROPE_BASE = 10000.0

_CACHE = {}


def _mm(nc, out, lhsT, rhs, **kw):
    nc.tensor.matmul(out, lhsT, rhs, **kw)


def build_program():
    nc = bacc.Bacc(name="csa_tp3")

    xt = nc.dram_tensor("xt", (C, T), F16, kind="ExternalInput")
    wqk = nc.dram_tensor("wqk", (C, NF * P), F16, kind="ExternalInput")
    bqk = nc.dram_tensor("bqk", (P, NF), F32, kind="ExternalInput")
    wv = nc.dram_tensor("wv", (C, HPC * D), F16, kind="ExternalInput")
    cs = nc.dram_tensor("cs", (P, T), F16, kind="ExternalInput")
    sw = nc.dram_tensor("sw", (P, T), F16, kind="ExternalInput")
    tri = nc.dram_tensor("tri", (P, P), F16, kind="ExternalInput")
    onesm = nc.dram_tensor("onesm", (P, P), F16, kind="ExternalInput")
    wp = nc.dram_tensor("wp", (HPC * D, C), F16, kind="ExternalInput")
    out = nc.dram_tensor("out", (T, C), F16, kind="ExternalOutput")

    v_spill = nc.dram_tensor("v_spill", (T, HPC * D), F16, kind="Internal")
    y_spill = nc.dram_tensor("y_spill", (HPC * D, T), F16, kind="Internal")

    with tile.TileContext(nc) as tc:
        with tc.tile_pool(name="persist", bufs=1) as persist:
            # q/k resident: 16 features x [128, T] fp16 (64 KB/partition)
            qk_res = [
                persist.tile([P, T], F16, tag=f"qk{f}", name=f"qk{f}")
                for f in range(NF)
            ]
            cs_t = persist.tile([P, T], F16, tag="cs", name="cs")
            sw_t = persist.tile([P, T], F16, tag="sw", name="sw")
            bqk_t = persist.tile([P, NF], F32, tag="bqk", name="bqk")
            tri_t = persist.tile([P, P], F16, tag="tri", name="tri")
            ones_t = persist.tile([P, P], F16, tag="ones", name="ones")

            with (
                tc.tile_pool(name="xt_res", bufs=1) as xt_res,
                tc.tile_pool(name="wq_pool", bufs=1) as wq_pool,
            ):
                xtt = [None] * CC  # [128, T] fp16 per c-chunk

                def load_wq(fg):
                    tiles = []
                    for c in range(CC):
                        w_ = wq_pool.tile([P, 2 * P], F16,
                                          tag=f"wq{c}", bufs=2,
                                          name=f"wq{c}")
                        nc.sync.dma_start(
                            w_[:],
                            wqk[c * P:(c + 1) * P,
                                fg * 2 * P:(fg + 1) * 2 * P],
                        )
                        tiles.append(w_)
                    return tiles

                # ---------------- phase A: V (two n-passes) --------------
                with (
                    tc.tile_pool(name="wv_pool", bufs=1) as wv_pool,
                    tc.tile_pool(name="va_pool", bufs=1) as va_pool,
                    tc.tile_pool(name="psum_a", bufs=1, space="PSUM") as psum_a,
                ):
                    # interleaved issue: (wva[c], xt_t0[c], wvb[c])
                    # triplets split in half-columns; t-outer consumption
                    # gives each later xt t-slice 8 chains of slack
                    wvh = [[None] * CC, [None] * CC]
                    HN = NT // 2
                    for c in range(CC):
                        wa = wv_pool.tile([P, NT], F16, tag=f"wva{c}",
                                          name=f"wva{c}")
                        wb = wv_pool.tile([P, NT], F16, tag=f"wvb{c}",
                                          name=f"wvb{c}")
                        x_ = xt_res.tile([P, T], F16, tag=f"x{c}",
                                         name=f"x{c}")
                        for hh in range(2):
                            hsl = slice(hh * HN, (hh + 1) * HN)
                            nc.sync.dma_start(
                                wa[:, hsl], wv[c * P:(c + 1) * P, hsl])
                            nc.sync.dma_start(
                                x_[:, hsl], xt[c * P:(c + 1) * P, hsl])
                            nc.sync.dma_start(
                                wb[:, hsl],
                                wv[c * P:(c + 1) * P,
                                   NT + hh * HN:NT + (hh + 1) * HN])
                        wvh[0][c] = wa
                        wvh[1][c] = wb
                        xtt[c] = x_
                    for tt_ in range(1, TT):
                        for c in range(CC):
                            nc.sync.dma_start(
                                xtt[c][:, tt_ * NT:(tt_ + 1) * NT],
                                xt[c * P:(c + 1) * P,
                                   tt_ * NT:(tt_ + 1) * NT])
                    # constants
                    nc.sync.dma_start(cs_t[:], cs[:])
                    nc.sync.dma_start(sw_t[:], sw[:])
                    nc.sync.dma_start(bqk_t[:], bqk[:])
                    nc.sync.dma_start(tri_t[:], tri[:])
                    nc.sync.dma_start(ones_t[:], onesm[:])
                    # prefetch first head's qk weights during phase A
                    wq_first = load_wq(0)

                    for tt_ in range(TT):
                        for n in range(2):
                            for m in range(4):
                                mtok = tt_ * 4 + m
                                msl = slice(mtok * P, (mtok + 1) * P)
                                ps = psum_a.tile([P, NT], F32, tag="psa",
                                                 bufs=8, name="psa")
                                for c in range(CC):
                                    _mm(nc, ps[:], xtt[c][:, msl],
                                        wvh[n][c][:],
                                        start=(c == 0), stop=(c == CC - 1))
                                vt = va_pool.tile([P, NT], F16, tag="vt",
                                                  bufs=8, name="vt")
                                nc.scalar.copy(vt[:], ps[:])
                                nc.gpsimd.dma_start(
                                    v_spill[mtok * P:(mtok + 1) * P,
                                            n * NT:(n + 1) * NT],
                                    vt[:],
                                )

                # ------------- merged phases B + C, per head -------------
                with (
                    tc.tile_pool(name="rp_pool", bufs=1) as rp_pool,
                    tc.tile_pool(name="vh_pool", bufs=1) as vh_pool,
                    tc.tile_pool(name="sd_pool", bufs=1) as sd_pool,
                    tc.tile_pool(name="psum_bc", bufs=1,
                                 space="PSUM") as psum_bc,
                ):
                    hd = D // 2

                    vh_t = [None] * HPC

                    def load_vh(h):
                        vh3 = vh_pool.tile([P, T // P, P], F16,
                                           tag="vh", bufs=3, name="vh3")
                        nc.sync.dma_start(
                            vh3[:],
                            v_spill[:, h * D:(h + 1) * D].rearrange(
                                "(j p) d -> p j d", p=P),
                        )
                        vh_t[h] = vh3

                    # C-block software pipeline (global across heads)
                    state = {}    # (h,t) -> (psy, p_sum)
                    pending = []  # [(h,t,j,nj,p,off)]
                    LOOK = 2

                    def c_front(h, t, j, nj):
                        qh = qk_res[2 * h]
                        kh = qk_res[2 * h + 1]
                        diag = (j >= 4 * t)
                        off = (j - 4 * t) * P if diag else 0
                        qsl = slice(t * NT + off, (t + 1) * NT)
                        pss = psum_bc.tile([P, NT], F32, tag="pss",
                                           bufs=3, name="pss")
                        _mm(nc, pss[:, off:],
                            kh[:, j * P:(j + 1) * P],
                            qh[:, qsl], start=True, stop=True)
                        p = sd_pool.tile([P, NT], F16, tag="p",
                                         bufs=5, name="p")
                        nc.scalar.activation(
                            p[:, off:], pss[:, off:], AF.Exp)
                        if diag:
                            nc.vector.tensor_mul(
                                p[:, off:off + P],
                                p[:, off:off + P],
                                tri_t[:],
                            )
                        if j == 0:
                            psy = psum_bc.tile([P, NT], F32, tag="psy",
                                               bufs=2, name="psy")
                            p_sum = sd_pool.tile([P, NT], F16,
                                                 tag="p_sum", bufs=2,
                                                 name="p_sum")
                            state[(h, t)] = (psy, p_sum)
                            nc.vector.tensor_copy(state[(h, t)][1][:], p[:])
                        else:
                            p_sum = state[(h, t)][1]
                            nc.vector.tensor_add(
                                p_sum[:, off:], p_sum[:, off:], p[:, off:])
                        pending.append((h, t, j, nj, p, off))

                    def c_back():
                        h, t, j, nj, p, off = pending.pop(0)
                        psy, p_sum = state[(h, t)]
                        _mm(nc, psy[:, off:],
                            vh_t[h][:, j, :], p[:, off:],
                            start=(j == 0), stop=(j == nj - 1))
                        if j == nj - 1:
                            psd = psum_bc.tile([P, NT], F32, tag="psd",
                                               bufs=1, name="psd")
                            _mm(nc, psd[:], ones_t[:], p_sum[:],
                                start=True, stop=True)
                            rden = sd_pool.tile([P, NT], F32,
                                                tag="rden", bufs=2,
                                                name="rden")
                            nc.vector.reciprocal_approx_fast(
                                rden[:], psd[:])
                            yst = sd_pool.tile([P, NT], F16,
                                                tag="yst", bufs=2,
                                                name="yst")
                            nc.vector.tensor_mul(yst[:], psy[:], rden[:])
                            nc.gpsimd.dma_start(
                                y_spill[h * P:(h + 1) * P,
                                        t * NT:(t + 1) * NT],
                                yst[:])
                            del state[(h, t)]

                    def chain(wq_t, h, f, t):
                        """One B-chain (16 mms) + RoPE for feature tile t."""
                        feat = h * 2 + f
                        ps = psum_bc.tile([P, NT], F32, tag="psb",
                                          bufs=2, name="psb")
                        for c in range(CC):
                            _mm(nc, ps[:],
                                wq_t[c][:, f * P:(f + 1) * P],
                                xtt[c][:, t * NT:(t + 1) * NT],
                                start=(c == 0), stop=(c == CC - 1))
                        sl = slice(t * NT, (t + 1) * NT)
                        raw = rp_pool.tile([P, NT], F16, tag="raw",
                                           bufs=2, name="raw")
                        nc.scalar.activation(
                            raw[:], ps[:], AF.Identity,
                            bias=bqk_t[:, feat:feat + 1],
                        )
                        rsw = rp_pool.tile([P, NT], F16, tag="rsw",
                                           bufs=2, name="rsw")
                        nc.scalar.activation(
                            rsw[0:hd, :], ps[hd:P, :], AF.Identity,
                            bias=bqk_t[hd:P, feat:feat + 1],
                        )
                        nc.scalar.activation(
                            rsw[hd:P, :], ps[0:hd, :], AF.Identity,
                            bias=bqk_t[0:hd, feat:feat + 1],
                        )
                        t1 = rp_pool.tile([P, NT], F16, tag="rt1",
                                          bufs=2, name="rt1")
                        t2 = rp_pool.tile([P, NT], F16, tag="rt2",
                                          bufs=2, name="rt2")
                        nc.vector.tensor_mul(t1[:], raw[:], cs_t[:, sl])
                        nc.vector.tensor_mul(t2[:], rsw[:], sw_t[:, sl])
                        nc.vector.tensor_add(
                            qk_res[feat][:, sl], t1[:], t2[:])

                    def c_group(h, t):
                        nj = 4 * t + 4
                        for j in range(nj):
                            c_front(h, t, j, nj)
                            if len(pending) > LOOK:
                                c_back()

                    wq_next = wq_first
                    load_vh(0)
                    load_vh(1)
                    for h in range(HPC):
                        # interleave qk-projection chains with SDPA groups
                        # so the tensor engine never waits on RoPE drains
                        wq_t = wq_next
                        if h + 1 < HPC:
                            wq_next = load_wq(h + 1)
                        if h + 2 < HPC:
                            load_vh(h + 2)
                        chain(wq_t, h, 1, 0)
                        chain(wq_t, h, 0, 0)
                        chain(wq_t, h, 0, 1)
                        chain(wq_t, h, 0, 2)
                        c_group(h, 0)
                        chain(wq_t, h, 0, 3)
                        chain(wq_t, h, 1, 1)
                        c_group(h, 1)
                        chain(wq_t, h, 1, 2)
                        c_group(h, 2)
                        chain(wq_t, h, 1, 3)
                        c_group(h, 3)
                    while pending:
                        c_back()

            # ------------- phase D: projection -------------
            with (
                tc.tile_pool(name="wp_pool", bufs=1) as wp_pool,
                tc.tile_pool(name="ym_pool", bufs=1) as ym_pool,
                tc.tile_pool(name="ot_pool", bufs=1) as ot_pool,
                tc.tile_pool(name="psum_d", bufs=1, space="PSUM") as psum_d,
            ):
                ym_t = [None] * (T // P)

                def load_ym(m):
                    ym = ym_pool.tile([P, HPC, P], F16, tag="ym",
                                      bufs=3, name="ym")
                    nc.sync.dma_start(
                        ym[:],
                        y_spill[:, m * P:(m + 1) * P].rearrange(
                            "(h d) t -> d h t", d=P),
                    )
                    ym_t[m] = ym

                load_ym(0)
                load_ym(1)
                wp_t = []
                for hh in range(HPC):
                    w_ = wp_pool.tile([P, C], F16, tag=f"wp{hh}",
                                      name=f"wp{hh}")
                    for n in range(4):
                        nc.sync.dma_start(
                            w_[:, n * NT:(n + 1) * NT],
                            wp[hh * P:(hh + 1) * P, n * NT:(n + 1) * NT])
                    wp_t.append(w_)
                for m in range(T // P):
                    if m + 2 < T // P:
                        load_ym(m + 2)
                    msl = slice(m * P, (m + 1) * P)
                    pso = [
                        psum_d.tile([P, NT], F32, tag=f"pso{n}",
                                    bufs=2, name=f"pso{n}")
                        for n in range(4)
                    ]
                    for hh in range(HPC):
                        lhsT = ym_t[m][:, hh, :]
                        for n in range(4):
                            _mm(nc, pso[n][:], lhsT,
                                wp_t[hh][:, n * NT:(n + 1) * NT],
                                start=(hh == 0),
                                stop=(hh == HPC - 1))
                    ot = ot_pool.tile([P, C], F16, tag="ot",
                                      bufs=2, name="ot")
                    for n in range(4):
                        nc.scalar.copy(
                            ot[:, n * NT:(n + 1) * NT], pso[n][:])
                        nc.gpsimd.dma_start(
                            out[msl, n * NT:(n + 1) * NT],
                            ot[:, n * NT:(n + 1) * NT])

    nc.finalize()
    return nc


def prep_inputs(x, w_attn, b_attn, w_proj, b_proj):
    """Build the 8 per-core input maps from full inputs."""
    x = np.asarray(x, dtype=np.float32)
    w_attn = np.asarray(w_attn, dtype=np.float32)
    b_attn = np.asarray(b_attn, dtype=np.float32)
    w_proj = np.asarray(w_proj, dtype=np.float32)

    scale = np.float32(1.0 / np.sqrt(D))

    inv_freq = 1.0 / (ROPE_BASE ** (np.arange(0, D, 2, dtype=np.float32) / D))
    tpos = np.arange(T, dtype=np.float32)
    ang = np.outer(tpos, inv_freq)  # [T, 64]
    cos_t, sin_t = np.cos(ang).T, np.sin(ang).T  # [64, T]
    cs = np.ascontiguousarray(
        np.concatenate([cos_t, cos_t], axis=0)).astype(np.float16)
    sw = np.ascontiguousarray(
        np.concatenate([-sin_t, sin_t], axis=0)).astype(np.float16)

    qq = np.arange(P)
    kk = np.arange(P)[:, None]
    tri = np.ascontiguousarray(
        (qq[None, :] >= kk).astype(np.float16))  # [128,128] causal triangle

    onesm = np.ones((P, P), dtype=np.float16)

    in_maps = []
    for core in range(8):
        b = core // 2
        hg = core % 2
        heads = list(range(hg * HPC, (hg + 1) * HPC))
        # interleaved feature order: (q_h, k_h) per head
        wqk_cols = []
        bqk_vals = []
        for h in heads:
            qcol = np.arange(h * D, (h + 1) * D)
            kcol = qcol + C
            wqk_cols.append(w_attn[:, qcol] * scale)
            wqk_cols.append(w_attn[:, kcol])
            bqk_vals.append(b_attn[qcol] * scale)
            bqk_vals.append(b_attn[kcol])
        wqk_s = np.ascontiguousarray(
            np.concatenate(wqk_cols, axis=1)).astype(np.float16)
        bqk_s = np.ascontiguousarray(
            np.stack(bqk_vals, axis=1)).astype(np.float32)  # [128, 16]

        vcols = np.concatenate(
            [np.arange(h * D, (h + 1) * D) for h in heads]) + 2 * C
        wv_s = np.ascontiguousarray(w_attn[:, vcols]).astype(np.float16)
        pcols = np.concatenate(
            [np.arange(h * D, (h + 1) * D) for h in heads])
        wp_s = np.ascontiguousarray(w_proj[pcols, :]).astype(np.float16)
        xt_s = np.ascontiguousarray(x[b].T).astype(np.float16)

        in_maps.append({
            "xt": xt_s, "wqk": wqk_s, "bqk": bqk_s, "wv": wv_s,
            "cs": cs, "sw": sw, "tri": tri, "onesm": onesm, "wp": wp_s,
        })
    return in_maps


def _get_program():
    if "nc" not in _CACHE:
        _CACHE["nc"] = build_program()
    return _CACHE["nc"]


def _postprocess(outs, b_proj, bvp):
    # bvp[hg]: bv_core @ wp_core for head-group hg — the attention value
    # bias contributes a token-independent row to the projection output.
    base = np.asarray(b_proj, dtype=np.float32) + bvp[0] + bvp[1]
    return np.stack(
        [outs[2 * b].astype(np.float32) + outs[2 * b + 1].astype(np.float32)
         + base[None, :] for b in range(B)]
    ).astype(np.float32)


def _run(inputs, trace=False):
    from concourse.bass_utils import run_bass_kernel_spmd

    nc = _get_program()
    in_maps = prep_inputs(
        inputs["x"], inputs["w_attn"], inputs["b_attn"],
        inputs["w_proj"], inputs["b_proj"],
    )
    b_attn = np.asarray(inputs["b_attn"], dtype=np.float32)
    w_proj = np.asarray(inputs["w_proj"], dtype=np.float32)
    bvp = []
    for hg in range(2):
        cols = np.concatenate(
            [np.arange(h * D, (h + 1) * D)
             for h in range(hg * HPC, (hg + 1) * HPC)])
        bvp.append(b_attn[2 * C + cols] @ w_proj[cols, :])
    res = run_bass_kernel_spmd(nc, in_maps, core_ids=list(range(8)),
                               trace=trace)
    full = _postprocess([r["out"] for r in res.results],
                        inputs["b_proj"], bvp)
    return full, res


def kernel(**inputs):
    full, _ = _run(inputs, trace=False)
    return full


if __name__ == "__main__":
    _get_program()
    print("built ok")



# revision 6
# speedup vs baseline: 1.0565x; 1.0565x over previous
"""Trainium2 Bass kernel for causal self-attention (B=4, T=2048, C=2048, H=16).

Sharding: 8 cores = 4 batches x 2 head-groups (8 heads each).
All-fp16 datapath (PSUM accumulation fp32). Per core:
  A) v = x @ Wv            -> fp16 spill [T, 1024]
  B+C merged, per head h:
    B-block: qkT features (q_h, k_h) = Wqk^T x^T + RoPE -> resident fp16
    C-block: flash-style SDPA, t-outer/j-inner, software-pipelined:
      scores mm -> exp (scalar) -> [diag tri-mul] -> p_sum += p (vector)
      -> PV mm accumulates numerator in PSUM (tensor, lagged 2 items)
      per t: ones-mm denominator from p_sum -> recip_approx (vector)
      -> y = psy * rden resident fp16
  D) partial_out = y^T @ wp -> [T, C] fp16 partial
Host sums core pairs per batch, adds b_proj and the folded bias row
bv @ wp (token-independent).

DMA layout: all inputs are host-repacked so that every DMA moves
multi-KB contiguous runs per partition (512B-row packets were the
phase-A bottleneck).  Phase A issues matmuls c-outer so all 8 psum
chains advance as each c-block of x/wv lands.  wp + ym are prefetched
during the last head's SDPA groups (xt/wq pools closed early).
"""

import sys
from contextlib import ExitStack

import numpy as np

sys.path.insert(0, "/opt/trn_rl_repo")

import concourse.bass as bass  # noqa: E402,F401
import concourse.mybir as mybir  # noqa: E402
import concourse.tile as tile  # noqa: E402
from concourse import bacc  # noqa: E402

F32 = mybir.dt.float32
F16 = mybir.dt.float16
AF = mybir.ActivationFunctionType

B, T, C = 4, 2048, 2048
H, D = 16, 128
HPC = 8            # heads per core
P = 128
NT = 512           # matmul moving free dim
TT = T // NT       # 4 token tiles
CC = C // P        # 16 contraction chunks over C
NF = 2 * HPC       # 16 feature chunks, interleaved (q_h, k_h) per head
ROPE_BASE = 10000.0

_CACHE = {}


def _mm(nc, out, lhsT, rhs, **kw):
    nc.tensor.matmul(out, lhsT, rhs, **kw)


def build_program():
    nc = bacc.Bacc(name="csa_tp4")

    xt = nc.dram_tensor("xt", (P, TT, CC, NT), F16, kind="ExternalInput")
    wqk = nc.dram_tensor("wqk", (P, HPC, CC, 2 * P), F16,
                         kind="ExternalInput")
    bqk = nc.dram_tensor("bqk", (P, NF), F32, kind="ExternalInput")
    wv = nc.dram_tensor("wv", (P, CC, HPC * D), F16, kind="ExternalInput")
    cs = nc.dram_tensor("cs", (P, T), F16, kind="ExternalInput")
    sw = nc.dram_tensor("sw", (P, T), F16, kind="ExternalInput")
    tri = nc.dram_tensor("tri", (P, P), F16, kind="ExternalInput")
    onesm = nc.dram_tensor("onesm", (P, P), F16, kind="ExternalInput")
    wp = nc.dram_tensor("wp", (P, HPC, C), F16, kind="ExternalInput")
    out = nc.dram_tensor("out", (T, C), F16, kind="ExternalOutput")

    v_spill = nc.dram_tensor("v_spill", (T, HPC * D), F16, kind="Internal")
    y_spill = nc.dram_tensor("y_spill", (HPC * D, T), F16, kind="Internal")

    with tile.TileContext(nc) as tc:
        with tc.tile_pool(name="persist", bufs=1) as persist:
            # q/k resident: 16 features x [128, T] fp16 (64 KB/partition)
            qk_res = [
                persist.tile([P, T], F16, tag=f"qk{f}", name=f"qk{f}")
                for f in range(NF)
            ]
            cs_t = persist.tile([P, T], F16, tag="cs", name="cs")
            sw_t = persist.tile([P, T], F16, tag="sw", name="sw")
            bqk_t = persist.tile([P, NF], F32, tag="bqk", name="bqk")
            tri_t = persist.tile([P, P], F16, tag="tri", name="tri")
            ones_t = persist.tile([P, P], F16, tag="ones", name="ones")

            # right-side stack: pools that close early (xt/wq) or open
            # late (phase-D wp/ym/ot) — keeps each side's LIFO order
            es_ax = ExitStack()
            xt_res = es_ax.enter_context(
                tc.tile_pool(name="xt_res", bufs=1, side="right"))
            wq_pool = es_ax.enter_context(
                tc.tile_pool(name="wq_pool", bufs=1, side="right"))

            xt_t = xt_res.tile([P, TT, CC, NT], F16, tag="xt", name="xt")

            def load_wq(h):
                w_ = wq_pool.tile([P, CC, 2 * P], F16, tag="wq", bufs=2,
                                  name="wq")
                nc.sync.dma_start(w_[:], wqk[:, h, :, :])
                return w_

            # ---------------- phase A: V ----------------
            with (
                tc.tile_pool(name="wv_pool", bufs=1) as wv_pool,
                tc.tile_pool(name="va_pool", bufs=1) as va_pool,
                tc.tile_pool(name="psum_a", bufs=1, space="PSUM") as psum_a,
            ):
                wv_t = wv_pool.tile([P, CC, HPC * D], F16, tag="wv",
                                    name="wv")
                # interleave wv / x(tt=0) blocks of 2 c-chunks so the
                # c-outer matmul issue below can start on block 0
                for cb in range(CC // 2):
                    csl = slice(2 * cb, 2 * cb + 2)
                    nc.sync.dma_start(wv_t[:, csl, :], wv[:, csl, :])
                    nc.sync.dma_start(xt_t[:, 0, csl, :], xt[:, 0, csl, :])
                for tt_ in range(1, TT):
                    for cb in range(4):
                        csl = slice(4 * cb, 4 * cb + 4)
                        nc.sync.dma_start(
                            xt_t[:, tt_, csl, :], xt[:, tt_, csl, :])
                # constants
                nc.sync.dma_start(cs_t[:], cs[:])
                nc.sync.dma_start(sw_t[:], sw[:])
                nc.sync.dma_start(bqk_t[:], bqk[:])
                nc.sync.dma_start(tri_t[:], tri[:])
                nc.sync.dma_start(ones_t[:], onesm[:])
                # prefetch first head's qk weights during phase A
                wq_first = load_wq(0)

                for tt_ in range(TT):
                    # c-outer: all 8 (n, m) psum chains advance per
                    # c-chunk, so the in-order tensor queue tracks the
                    # DMA block arrivals instead of blocking on chain 0
                    pst = [
                        psum_a.tile([P, NT], F32, tag=f"psa{nm}",
                                    bufs=1, name=f"psa{nm}")
                        for nm in range(8)
                    ]
                    for c in range(CC):
                        for nm in range(8):
                            n, m = nm // 4, nm % 4
                            _mm(nc, pst[nm][:],
                                xt_t[:, tt_, c, m * P:(m + 1) * P],
                                wv_t[:, c, n * NT:(n + 1) * NT],
                                start=(c == 0), stop=(c == CC - 1))
                    for nm in range(8):
                        n, m = nm // 4, nm % 4
                        mtok = tt_ * 4 + m
                        vt = va_pool.tile([P, NT], F16, tag=f"vt{nm}",
                                          bufs=2, name=f"vt{nm}")
                        nc.scalar.copy(vt[:], pst[nm][:])
                        nc.gpsimd.dma_start(
                            v_spill[mtok * P:(mtok + 1) * P,
                                    n * NT:(n + 1) * NT],
                            vt[:],
                        )

            # ------------- merged phases B + C, per head -------------
            es_bc = ExitStack()
            rp_pool = es_bc.enter_context(
                tc.tile_pool(name="rp_pool", bufs=1))
            vh_pool = es_bc.enter_context(
                tc.tile_pool(name="vh_pool", bufs=1))
            sd_pool = es_bc.enter_context(
                tc.tile_pool(name="sd_pool", bufs=1))
            psum_bc = es_bc.enter_context(
                tc.tile_pool(name="psum_bc", bufs=1, space="PSUM"))

            es_d = ExitStack()

            hd = D // 2

            vh_t = [None] * HPC

            def load_vh(h):
                vh3 = vh_pool.tile([P, T // P, P], F16,
                                   tag="vh", bufs=3, name="vh3")
                nc.sync.dma_start(
                    vh3[:],
                    v_spill[:, h * D:(h + 1) * D].rearrange(
                        "(j p) d -> p j d", p=P),
                )
                vh_t[h] = vh3

            # C-block software pipeline (global across heads)
            state = {}    # (h,t) -> (psy, p_sum)
            pending = []  # [(h,t,j,nj,p,off)]
            LOOK = 2

            def c_front(h, t, j, nj):
                qh = qk_res[2 * h]
                kh = qk_res[2 * h + 1]
                diag = (j >= 4 * t)
                off = (j - 4 * t) * P if diag else 0
                qsl = slice(t * NT + off, (t + 1) * NT)
                pss = psum_bc.tile([P, NT], F32, tag="pss",
                                   bufs=3, name="pss")
                _mm(nc, pss[:, off:],
                    kh[:, j * P:(j + 1) * P],
                    qh[:, qsl], start=True, stop=True)
                p = sd_pool.tile([P, NT], F16, tag="p",
                                 bufs=5, name="p")
                nc.scalar.activation(
                    p[:, off:], pss[:, off:], AF.Exp)
                if diag:
                    nc.vector.tensor_mul(
                        p[:, off:off + P],
                        p[:, off:off + P],
                        tri_t[:],
                    )
                if j == 0:
                    psy = psum_bc.tile([P, NT], F32, tag="psy",
                                       bufs=2, name="psy")
                    p_sum = sd_pool.tile([P, NT], F16,
                                         tag="p_sum", bufs=2,
                                         name="p_sum")
                    state[(h, t)] = (psy, p_sum)
                    nc.vector.tensor_copy(state[(h, t)][1][:], p[:])
                else:
                    p_sum = state[(h, t)][1]
                    nc.vector.tensor_add(
                        p_sum[:, off:], p_sum[:, off:], p[:, off:])
                pending.append((h, t, j, nj, p, off))

            def c_back():
                h, t, j, nj, p, off = pending.pop(0)
                psy, p_sum = state[(h, t)]
                _mm(nc, psy[:, off:],
                    vh_t[h][:, j, :], p[:, off:],
                    start=(j == 0), stop=(j == nj - 1))
                if j == nj - 1:
                    psd = psum_bc.tile([P, NT], F32, tag="psd",
                                       bufs=1, name="psd")
                    _mm(nc, psd[:], ones_t[:], p_sum[:],
                        start=True, stop=True)
                    rden = sd_pool.tile([P, NT], F32,
                                        tag="rden", bufs=2,
                                        name="rden")
                    nc.vector.reciprocal_approx_fast(
                        rden[:], psd[:])
                    yst = sd_pool.tile([P, NT], F16,
                                       tag="yst", bufs=2,
                                       name="yst")
                    nc.vector.tensor_mul(yst[:], psy[:], rden[:])
                    nc.gpsimd.dma_start(
                        y_spill[h * P:(h + 1) * P,
                                t * NT:(t + 1) * NT],
                        yst[:])
                    del state[(h, t)]

            def chain(wq_t, h, f, t):
                """One B-chain (16 mms) + RoPE for feature tile t."""
                feat = h * 2 + f
                ps = psum_bc.tile([P, NT], F32, tag="psb",
                                  bufs=2, name="psb")
                for c in range(CC):
                    _mm(nc, ps[:],
                        wq_t[:, c, f * P:(f + 1) * P],
                        xt_t[:, t, c, :],
                        start=(c == 0), stop=(c == CC - 1))
                sl = slice(t * NT, (t + 1) * NT)
                raw = rp_pool.tile([P, NT], F16, tag="raw",
                                   bufs=2, name="raw")
                nc.scalar.activation(
                    raw[:], ps[:], AF.Identity,
                    bias=bqk_t[:, feat:feat + 1],
                )
                rsw = rp_pool.tile([P, NT], F16, tag="rsw",
                                   bufs=2, name="rsw")
                nc.scalar.activation(
                    rsw[0:hd, :], ps[hd:P, :], AF.Identity,
                    bias=bqk_t[hd:P, feat:feat + 1],
                )
                nc.scalar.activation(
                    rsw[hd:P, :], ps[0:hd, :], AF.Identity,
                    bias=bqk_t[0:hd, feat:feat + 1],
                )
                t1 = rp_pool.tile([P, NT], F16, tag="rt1",
                                  bufs=2, name="rt1")
                t2 = rp_pool.tile([P, NT], F16, tag="rt2",
                                  bufs=2, name="rt2")
                nc.vector.tensor_mul(t1[:], raw[:], cs_t[:, sl])
                nc.vector.tensor_mul(t2[:], rsw[:], sw_t[:, sl])
                nc.vector.tensor_add(
                    qk_res[feat][:, sl], t1[:], t2[:])

            def c_group(h, t):
                nj = 4 * t + 4
                for j in range(nj):
                    c_front(h, t, j, nj)
                    if len(pending) > LOOK:
                        c_back()

            ym_t = [None] * (T // P)
            dpools = {}

            def load_ym(m):
                ym = dpools["ym"].tile([P, HPC, P], F16, tag="ym",
                                       bufs=6, name="ym")
                nc.sync.dma_start(
                    ym[:],
                    y_spill[:, m * P:(m + 1) * P].rearrange(
                        "(h d) t -> d h t", d=P),
                )
                ym_t[m] = ym

            wq_next = wq_first
            load_vh(0)
            load_vh(1)
            for h in range(HPC):
                # interleave qk-projection chains with SDPA groups
                # so the tensor engine never waits on RoPE drains
                wq_t = wq_next
                if h + 1 < HPC:
                    wq_next = load_wq(h + 1)
                if h + 2 < HPC:
                    load_vh(h + 2)
                chain(wq_t, h, 1, 0)
                chain(wq_t, h, 0, 0)
                chain(wq_t, h, 0, 1)
                chain(wq_t, h, 0, 2)
                c_group(h, 0)
                chain(wq_t, h, 0, 3)
                chain(wq_t, h, 1, 1)
                c_group(h, 1)
                chain(wq_t, h, 1, 2)
                if h < HPC - 1:
                    c_group(h, 2)
                    chain(wq_t, h, 1, 3)
                    c_group(h, 3)
                else:
                    # last head: finish all chains, free the x / wq
                    # pools, prefetch phase-D weights + first y tiles
                    # under the remaining SDPA groups
                    chain(wq_t, h, 1, 3)
                    es_ax.close()
                    wp_pool = es_d.enter_context(
                        tc.tile_pool(name="wp_pool", bufs=1,
                                     side="right"))
                    ym_pool = es_d.enter_context(
                        tc.tile_pool(name="ym_pool", bufs=1,
                                     side="right"))
                    ot_pool = es_d.enter_context(
                        tc.tile_pool(name="ot_pool", bufs=1,
                                     side="right"))
                    dpools["wp"] = wp_pool
                    dpools["ym"] = ym_pool
                    dpools["ot"] = ot_pool
                    wp_t = wp_pool.tile([P, HPC, C], F16, tag="wp",
                                        name="wp")
                    dpools["wp_t"] = wp_t
                    for hb in range(2):
                        nc.sync.dma_start(
                            wp_t[:, 4 * hb:4 * hb + 4, :],
                            wp[:, 4 * hb:4 * hb + 4, :])
                    for m0 in range(4):
                        load_ym(m0)
                    c_group(h, 2)
                    c_group(h, 3)
            while pending:
                c_back()
            es_bc.close()

            # ------------- phase D: projection -------------
            with tc.tile_pool(name="psum_d", bufs=1,
                              space="PSUM") as psum_d:
                wpt = dpools["wp_t"]
                ot_pool = dpools["ot"]
                for m in range(T // P):
                    if m + 4 < T // P:
                        load_ym(m + 4)
                    msl = slice(m * P, (m + 1) * P)
                    pso = [
                        psum_d.tile([P, NT], F32, tag=f"pso{n}",
                                    bufs=2, name=f"pso{n}")
                        for n in range(4)
                    ]
                    for hh in range(HPC):
                        lhsT = ym_t[m][:, hh, :]
                        for n in range(4):
                            _mm(nc, pso[n][:], lhsT,
                                wpt[:, hh, n * NT:(n + 1) * NT],
                                start=(hh == 0),
                                stop=(hh == HPC - 1))
                    ot = ot_pool.tile([P, C], F16, tag="ot",
                                      bufs=2, name="ot")
                    for n in range(4):
                        nc.scalar.copy(
                            ot[:, n * NT:(n + 1) * NT], pso[n][:])
                        nc.gpsimd.dma_start(
                            out[msl, n * NT:(n + 1) * NT],
                            ot[:, n * NT:(n + 1) * NT])
            es_d.close()

    nc.finalize()
    return nc


def prep_inputs(x, w_attn, b_attn, w_proj, b_proj):
    """Build the 8 per-core input maps from full inputs.

    All tensors are repacked so SBUF partition rows are contiguous
    multi-KB runs in DRAM (fast DMA packets):
      xt  [P, TT, CC, NT]: xt[p,tt,c,n]  = x[tt*NT+n, c*P+p]
      wqk [P, HPC, CC, 2P]: per head-pair block, c-major
      wv  [P, CC, HPC*D]
      wp  [P, HPC, C]
    """
    x = np.asarray(x, dtype=np.float32)
    w_attn = np.asarray(w_attn, dtype=np.float32)
    b_attn = np.asarray(b_attn, dtype=np.float32)
    w_proj = np.asarray(w_proj, dtype=np.float32)

    scale = np.float32(1.0 / np.sqrt(D))

    inv_freq = 1.0 / (ROPE_BASE ** (np.arange(0, D, 2, dtype=np.float32) / D))
    tpos = np.arange(T, dtype=np.float32)
    ang = np.outer(tpos, inv_freq)  # [T, 64]
    cos_t, sin_t = np.cos(ang).T, np.sin(ang).T  # [64, T]
    cs = np.ascontiguousarray(
        np.concatenate([cos_t, cos_t], axis=0)).astype(np.float16)
    sw = np.ascontiguousarray(
        np.concatenate([-sin_t, sin_t], axis=0)).astype(np.float16)

    qq = np.arange(P)
    kk = np.arange(P)[:, None]
    tri = np.ascontiguousarray(
        (qq[None, :] >= kk).astype(np.float16))  # [128,128] causal triangle

    onesm = np.ones((P, P), dtype=np.float16)

    in_maps = []
    for core in range(8):
        b = core // 2
        hg = core % 2
        heads = list(range(hg * HPC, (hg + 1) * HPC))
        # interleaved feature order: (q_h, k_h) per head
        wqk_cols = []
        bqk_vals = []
        for h in heads:
            qcol = np.arange(h * D, (h + 1) * D)
            kcol = qcol + C
            wqk_cols.append(w_attn[:, qcol] * scale)
            wqk_cols.append(w_attn[:, kcol])
            bqk_vals.append(b_attn[qcol] * scale)
            bqk_vals.append(b_attn[kcol])
        wqk_full = np.concatenate(wqk_cols, axis=1)  # [C, NF*P]
        # -> [P, HPC, CC, 2P]
        wqk_s = np.ascontiguousarray(
            wqk_full.reshape(CC, P, HPC, 2 * P).transpose(1, 2, 0, 3)
        ).astype(np.float16)
        bqk_s = np.ascontiguousarray(
            np.stack(bqk_vals, axis=1)).astype(np.float32)  # [128, 16]

        vcols = np.concatenate(
            [np.arange(h * D, (h + 1) * D) for h in heads]) + 2 * C
        wv_full = w_attn[:, vcols]  # [C, HPC*D]
        wv_s = np.ascontiguousarray(
            wv_full.reshape(CC, P, HPC * D).transpose(1, 0, 2)
        ).astype(np.float16)
        pcols = np.concatenate(
            [np.arange(h * D, (h + 1) * D) for h in heads])
        wp_full = w_proj[pcols, :]  # [HPC*D, C]
        wp_s = np.ascontiguousarray(
            wp_full.reshape(HPC, P, C).transpose(1, 0, 2)
        ).astype(np.float16)
        # x: [T, C] -> [P, TT, CC, NT]
        xt_s = np.ascontiguousarray(
            x[b].T.reshape(CC, P, TT, NT).transpose(1, 2, 0, 3)
        ).astype(np.float16)

        in_maps.append({
            "xt": xt_s, "wqk": wqk_s, "bqk": bqk_s, "wv": wv_s,
            "cs": cs, "sw": sw, "tri": tri, "onesm": onesm, "wp": wp_s,
        })
    return in_maps


def _get_program():
    if "nc" not in _CACHE:
        _CACHE["nc"] = build_program()
    return _CACHE["nc"]


def _postprocess(outs, b_proj, bvp):
    # bvp[hg]: bv_core @ wp_core for head-group hg — the attention value
    # bias contributes a token-independent row to the projection output.
    base = np.asarray(b_proj, dtype=np.float32) + bvp[0] + bvp[1]
    return np.stack(
        [outs[2 * b].astype(np.float32) + outs[2 * b + 1].astype(np.float32)
         + base[None, :] for b in range(B)]
    ).astype(np.float32)


def _run(inputs, trace=False):
    from concourse.bass_utils import run_bass_kernel_spmd

    nc = _get_program()
    in_maps = prep_inputs(
        inputs["x"], inputs["w_attn"], inputs["b_attn"],
        inputs["w_proj"], inputs["b_proj"],
    )
    b_attn = np.asarray(inputs["b_attn"], dtype=np.float32)
    w_proj = np.asarray(inputs["w_proj"], dtype=np.float32)
    bvp = []
    for hg in range(2):
        cols = np.concatenate(
            [np.arange(h * D, (h + 1) * D)
             for h in range(hg * HPC, (hg + 1) * HPC)])
        bvp.append(b_attn[2 * C + cols] @ w_proj[cols, :])
    res = run_bass_kernel_spmd(nc, in_maps, core_ids=list(range(8)),
                               trace=trace)
    full = _postprocess([r["out"] for r in res.results],
                        inputs["b_proj"], bvp)
    return full, res


def kernel(**inputs):
    full, _ = _run(inputs, trace=False)
    return full


if __name__ == "__main__":
    _get_program()
    print("built ok")


# revision 19
# speedup vs baseline: 1.0573x; 1.0008x over previous
"""Trainium2 Bass kernel for causal self-attention (B=4, T=2048, C=2048, H=16).

Sharding: 8 cores = 4 batches x 2 head-groups (8 heads each).
All-fp16 datapath (PSUM accumulation fp32). Per core:
  A) v = x @ Wv            -> fp16 spill [T, 1024]
  B+C merged, per head h:
    B-block: qkT features (q_h, k_h) = Wqk^T x^T + RoPE -> resident fp16
    C-block: flash-style SDPA, t-outer/j-inner, software-pipelined:
      scores mm -> exp (scalar) -> [diag tri-mul] -> p_sum += p (vector)
      -> PV mm accumulates numerator in PSUM (tensor, lagged 2 items)
      per t: ones-mm denominator from p_sum -> recip_approx (vector)
      -> y = psy * rden resident fp16
  D) partial_out = y^T @ wp -> [T, C] fp16 partial
Host sums core pairs per batch, adds b_proj and the folded bias row
bv @ wp (token-independent).

DMA layout: all inputs are host-repacked so that every DMA moves
multi-KB contiguous runs per partition (512B-row packets were the
phase-A bottleneck).  Phase A issues matmuls c-outer so all 8 psum
chains advance as each c-block of x/wv lands.  wp + ym are prefetched
during the last head's SDPA groups (xt/wq pools closed early).
"""

import sys
from contextlib import ExitStack

import numpy as np

sys.path.insert(0, "/opt/trn_rl_repo")

import concourse.bass as bass  # noqa: E402,F401
import concourse.mybir as mybir  # noqa: E402
import concourse.tile as tile  # noqa: E402
from concourse import bacc  # noqa: E402

F32 = mybir.dt.float32
F16 = mybir.dt.float16
AF = mybir.ActivationFunctionType

B, T, C = 4, 2048, 2048
H, D = 16, 128
HPC = 8            # heads per core
P = 128
NT = 512           # matmul moving free dim
TT = T // NT       # 4 token tiles
CC = C // P        # 16 contraction chunks over C
NF = 2 * HPC       # 16 feature chunks, interleaved (q_h, k_h) per head
ROPE_BASE = 10000.0

_CACHE = {}


def _mm(nc, out, lhsT, rhs, **kw):
    nc.tensor.matmul(out, lhsT, rhs, **kw)


def build_program():
    nc = bacc.Bacc(name="csa_tp4")

    xt = nc.dram_tensor("xt", (P, TT, CC, NT), F16, kind="ExternalInput")
    wqk = nc.dram_tensor("wqk", (P, HPC, CC, 2 * P), F16,
                         kind="ExternalInput")
    bqk = nc.dram_tensor("bqk", (P, NF), F32, kind="ExternalInput")
    wv = nc.dram_tensor("wv", (P, CC, HPC * D), F16, kind="ExternalInput")
    cs = nc.dram_tensor("cs", (P, T), F16, kind="ExternalInput")
    sw = nc.dram_tensor("sw", (P, T), F16, kind="ExternalInput")
    tri = nc.dram_tensor("tri", (P, P), F16, kind="ExternalInput")
    onesm = nc.dram_tensor("onesm", (P, P), F16, kind="ExternalInput")
    wp = nc.dram_tensor("wp", (P, HPC, C), F16, kind="ExternalInput")
    out = nc.dram_tensor("out", (T, C), F16, kind="ExternalOutput")

    v_spill = nc.dram_tensor("v_spill", (T, HPC * D), F16, kind="Internal")
    y_spill = nc.dram_tensor("y_spill", (HPC * D, T), F16, kind="Internal")

    with tile.TileContext(nc) as tc:
        with tc.tile_pool(name="persist", bufs=1) as persist:
            # q/k resident: 16 features x [128, T] fp16 (64 KB/partition)
            qk_res = [
                persist.tile([P, T], F16, tag=f"qk{f}", name=f"qk{f}")
                for f in range(NF)
            ]
            cs_t = persist.tile([P, T], F16, tag="cs", name="cs")
            sw_t = persist.tile([P, T], F16, tag="sw", name="sw")
            bqk_t = persist.tile([P, NF], F32, tag="bqk", name="bqk")
            tri_t = persist.tile([P, P], F16, tag="tri", name="tri")
            ones_t = persist.tile([P, P], F16, tag="ones", name="ones")
            # t=0 y tiles captured SBUF->SBUF (no DRAM roundtrip), so
            # phase D can start on m=0..3 without racing y_spill writes
            ym_early = [
                persist.tile([P, HPC, P], F16, tag=f"yme{m}",
                             name=f"yme{m}")
                for m in range(4)
            ]

            # right-side stack: pools that close early (xt/wq) or open
            # late (phase-D wp/ym/ot) — keeps each side's LIFO order
            es_ax = ExitStack()
            xt_res = es_ax.enter_context(
                tc.tile_pool(name="xt_res", bufs=1, side="right"))
            wq_pool = es_ax.enter_context(
                tc.tile_pool(name="wq_pool", bufs=1, side="right"))

            xt_t = xt_res.tile([P, TT, CC, NT], F16, tag="xt", name="xt")

            def load_wq(h):
                w_ = wq_pool.tile([P, CC, 2 * P], F16, tag="wq", bufs=2,
                                  name="wq")
                nc.sync.dma_start(w_[:], wqk[:, h, :, :])
                return w_

            # ---------------- phase A: V ----------------
            with (
                tc.tile_pool(name="wv_pool", bufs=1) as wv_pool,
                tc.tile_pool(name="va_pool", bufs=1) as va_pool,
                tc.tile_pool(name="psum_a", bufs=1, space="PSUM") as psum_a,
            ):
                wv_t = wv_pool.tile([P, CC, HPC * D], F16, tag="wv",
                                    name="wv")
                # split wv / x(tt=0) per-chunk loads across the two
                # HWDGE queues (sync + scalar) so the c-outer matmul
                # issue below tracks arrivals from chunk 0
                for i in range(CC // 2):
                    c0, c1 = 2 * i, 2 * i + 1
                    nc.sync.dma_start(wv_t[:, c0, :], wv[:, c0, :])
                    nc.scalar.dma_start(wv_t[:, c1, :], wv[:, c1, :])
                    nc.sync.dma_start(xt_t[:, 0, c0, :], xt[:, 0, c0, :])
                    nc.scalar.dma_start(xt_t[:, 0, c1, :], xt[:, 0, c1, :])
                for cb in range(4):
                    csl = slice(4 * cb, 4 * cb + 4)
                    nc.sync.dma_start(xt_t[:, 1, csl, :], xt[:, 1, csl, :])
                # constants + first head's qk weights on the scalar queue
                nc.scalar.dma_start(cs_t[:], cs[:])
                nc.scalar.dma_start(sw_t[:], sw[:])
                nc.scalar.dma_start(bqk_t[:], bqk[:])
                nc.scalar.dma_start(tri_t[:], tri[:])
                nc.scalar.dma_start(ones_t[:], onesm[:])
                wq_first = wq_pool.tile([P, CC, 2 * P], F16, tag="wq",
                                        bufs=2, name="wq")
                nc.scalar.dma_start(wq_first[:], wqk[:, 0, :, :])
                for cb in range(4):
                    csl = slice(4 * cb, 4 * cb + 4)
                    nc.scalar.dma_start(
                        xt_t[:, 2, csl, :], xt[:, 2, csl, :])
                for cb in range(4):
                    csl = slice(4 * cb, 4 * cb + 4)
                    nc.sync.dma_start(xt_t[:, 3, csl, :], xt[:, 3, csl, :])

                for tt_ in range(TT):
                    # c-outer: all 8 (n, m) psum chains advance per
                    # c-chunk, so the in-order tensor queue tracks the
                    # DMA block arrivals instead of blocking on chain 0
                    pst = [
                        psum_a.tile([P, NT], F32, tag=f"psa{nm}",
                                    bufs=1, name=f"psa{nm}")
                        for nm in range(8)
                    ]
                    for c in range(CC):
                        for nm in range(8):
                            n, m = nm // 4, nm % 4
                            _mm(nc, pst[nm][:],
                                xt_t[:, tt_, c, m * P:(m + 1) * P],
                                wv_t[:, c, n * NT:(n + 1) * NT],
                                start=(c == 0), stop=(c == CC - 1))
                    for nm in range(8):
                        n, m = nm // 4, nm % 4
                        mtok = tt_ * 4 + m
                        vt = va_pool.tile([P, NT], F16, tag=f"vt{nm}",
                                          bufs=1, name=f"vt{nm}")
                        nc.scalar.copy(vt[:], pst[nm][:])
                        nc.gpsimd.dma_start(
                            v_spill[mtok * P:(mtok + 1) * P,
                                    n * NT:(n + 1) * NT],
                            vt[:],
                        )

            # ------------- merged phases B + C, per head -------------
            es_bc = ExitStack()
            rp_pool = es_bc.enter_context(
                tc.tile_pool(name="rp_pool", bufs=1))
            vh_pool = es_bc.enter_context(
                tc.tile_pool(name="vh_pool", bufs=1))
            sd_pool = es_bc.enter_context(
                tc.tile_pool(name="sd_pool", bufs=1))
            psum_bc = es_bc.enter_context(
                tc.tile_pool(name="psum_bc", bufs=1, space="PSUM"))

            es_d = ExitStack()

            hd = D // 2

            vh_t = [None] * HPC

            def load_vh(h):
                vh3 = vh_pool.tile([P, T // P, P], F16,
                                   tag="vh", bufs=3, name="vh3")
                nc.sync.dma_start(
                    vh3[:],
                    v_spill[:, h * D:(h + 1) * D].rearrange(
                        "(j p) d -> p j d", p=P),
                )
                vh_t[h] = vh3

            # C-block software pipeline (global across heads)
            state = {}    # (h,t) -> (psy, p_sum)
            pending = []  # [(h,t,j,nj,p,off)]
            LOOK = 2

            def c_front(h, t, j, nj):
                qh = qk_res[2 * h]
                kh = qk_res[2 * h + 1]
                diag = (j >= 4 * t)
                off = (j - 4 * t) * P if diag else 0
                qsl = slice(t * NT + off, (t + 1) * NT)
                pss = psum_bc.tile([P, NT], F32, tag="pss",
                                   bufs=3, name="pss")
                _mm(nc, pss[:, off:],
                    kh[:, j * P:(j + 1) * P],
                    qh[:, qsl], start=True, stop=True)
                p = sd_pool.tile([P, NT], F16, tag="p",
                                 bufs=5, name="p")
                nc.scalar.activation(
                    p[:, off:], pss[:, off:], AF.Exp)
                if diag:
                    nc.vector.tensor_mul(
                        p[:, off:off + P],
                        p[:, off:off + P],
                        tri_t[:],
                    )
                if j == 0:
                    psy = psum_bc.tile([P, NT], F32, tag="psy",
                                       bufs=2, name="psy")
                    p_sum = sd_pool.tile([P, NT], F16,
                                         tag="p_sum", bufs=2,
                                         name="p_sum")
                    state[(h, t)] = (psy, p_sum)
                    nc.vector.tensor_copy(state[(h, t)][1][:], p[:])
                else:
                    p_sum = state[(h, t)][1]
                    nc.vector.tensor_add(
                        p_sum[:, off:], p_sum[:, off:], p[:, off:])
                pending.append((h, t, j, nj, p, off))

            def c_back():
                h, t, j, nj, p, off = pending.pop(0)
                psy, p_sum = state[(h, t)]
                _mm(nc, psy[:, off:],
                    vh_t[h][:, j, :], p[:, off:],
                    start=(j == 0), stop=(j == nj - 1))
                if j == nj - 1:
                    psd = psum_bc.tile([P, NT], F32, tag="psd",
                                       bufs=1, name="psd")
                    _mm(nc, psd[:], ones_t[:], p_sum[:],
                        start=True, stop=True)
                    rden = sd_pool.tile([P, NT], F32,
                                        tag="rden", bufs=2,
                                        name="rden")
                    nc.vector.reciprocal_approx_fast(
                        rden[:], psd[:])
                    yst = sd_pool.tile([P, NT], F16,
                                       tag="yst", bufs=2,
                                       name="yst")
                    nc.vector.tensor_mul(yst[:], psy[:], rden[:])
                    if t == 0:
                        # capture in SBUF for the early phase-D tiles
                        for m in range(4):
                            nc.sync.dma_start(
                                ym_early[m][:, h, :],
                                yst[:, m * P:(m + 1) * P])
                    else:
                        nc.gpsimd.dma_start(
                            y_spill[h * P:(h + 1) * P,
                                    t * NT:(t + 1) * NT],
                            yst[:])
                    del state[(h, t)]

            def chain(wq_t, h, f, t):
                """One B-chain (16 mms) + RoPE for feature tile t."""
                feat = h * 2 + f
                ps = psum_bc.tile([P, NT], F32, tag="psb",
                                  bufs=2, name="psb")
                for c in range(CC):
                    _mm(nc, ps[:],
                        wq_t[:, c, f * P:(f + 1) * P],
                        xt_t[:, t, c, :],
                        start=(c == 0), stop=(c == CC - 1))
                sl = slice(t * NT, (t + 1) * NT)
                raw = rp_pool.tile([P, NT], F16, tag="raw",
                                   bufs=2, name="raw")
                nc.scalar.activation(
                    raw[:], ps[:], AF.Identity,
                    bias=bqk_t[:, feat:feat + 1],
                )
                rsw = rp_pool.tile([P, NT], F16, tag="rsw",
                                   bufs=2, name="rsw")
                nc.scalar.activation(
                    rsw[0:hd, :], ps[hd:P, :], AF.Identity,
                    bias=bqk_t[hd:P, feat:feat + 1],
                )
                nc.scalar.activation(
                    rsw[hd:P, :], ps[0:hd, :], AF.Identity,
                    bias=bqk_t[0:hd, feat:feat + 1],
                )
                t1 = rp_pool.tile([P, NT], F16, tag="rt1",
                                  bufs=2, name="rt1")
                t2 = rp_pool.tile([P, NT], F16, tag="rt2",
                                  bufs=2, name="rt2")
                nc.vector.tensor_mul(t1[:], raw[:], cs_t[:, sl])
                nc.vector.tensor_mul(t2[:], rsw[:], sw_t[:, sl])
                nc.vector.tensor_add(
                    qk_res[feat][:, sl], t1[:], t2[:])

            def c_group(h, t, filler=None):
                nj = 4 * t + 4
                for j in range(nj):
                    c_front(h, t, j, nj)
                    if filler is not None:
                        filler()
                        filler()
                    if len(pending) > LOOK:
                        c_back()

            ym_t = [None] * (T // P)
            for m in range(4):
                ym_t[m] = ym_early[m]
            dpools = {}

            def load_ym(m, eng=None):
                ym = dpools["ym"].tile([P, HPC, P], F16, tag="ym",
                                       bufs=6, name="ym")
                (eng or nc.sync).dma_start(
                    ym[:],
                    y_spill[:, m * P:(m + 1) * P].rearrange(
                        "(h d) t -> d h t", d=P),
                )
                ym_t[m] = ym

            def d_tile_gen(m, psum_pool, tagf, psum_bufs, copy_vec):
                """Yield after each tensor op of output tile m.

                hh-outer over n-pairs: wp hh-blocks are consumed in
                DMA arrival order, and each ym[hh] stationary feeds
                two consecutive matmuls.
                """
                msl = slice(m * P, (m + 1) * P)
                ot = dpools["ot"].tile([P, C], F16, tag="ot", bufs=2,
                                       name="ot")
                wpt = dpools["wp_t"]
                for half in range(2):
                    pr = [
                        psum_pool.tile([P, NT], F32,
                                       tag=tagf(2 * half + k),
                                       bufs=psum_bufs, name="psD")
                        for k in range(2)
                    ]
                    for hh in range(HPC):
                        for k in range(2):
                            n = 2 * half + k
                            _mm(nc, pr[k][:], ym_t[m][:, hh, :],
                                wpt[:, hh, n * NT:(n + 1) * NT],
                                start=(hh == 0), stop=(hh == HPC - 1))
                            yield
                    for k in range(2):
                        n = 2 * half + k
                        nsl = slice(n * NT, (n + 1) * NT)
                        if copy_vec:
                            nc.vector.tensor_copy(ot[:, nsl], pr[k][:])
                        else:
                            nc.scalar.copy(ot[:, nsl], pr[k][:])
                        nc.gpsimd.dma_start(out[msl, nsl], ot[:, nsl])
                        yield

            wq_next = wq_first
            load_vh(0)
            load_vh(1)
            for h in range(HPC):
                # interleave qk-projection chains with SDPA groups
                # so the tensor engine never waits on RoPE drains
                wq_t = wq_next
                if h + 1 < HPC:
                    wq_next = load_wq(h + 1)
                if h + 2 < HPC:
                    load_vh(h + 2)
                if h < HPC - 1:
                    chain(wq_t, h, 1, 0)
                    chain(wq_t, h, 0, 0)
                    chain(wq_t, h, 0, 1)
                    chain(wq_t, h, 0, 2)
                    c_group(h, 0)
                    chain(wq_t, h, 0, 3)
                    chain(wq_t, h, 1, 1)
                    c_group(h, 1)
                    chain(wq_t, h, 1, 2)
                    c_group(h, 2)
                    chain(wq_t, h, 1, 3)
                    c_group(h, 3)
                else:
                    # last head: all chains first, then free the x / wq
                    # pools and prefetch phase-D weights + first y
                    # tiles; interleave the first two phase-D output
                    # tiles into the exp-bound SDPA groups
                    chain(wq_t, h, 1, 0)
                    chain(wq_t, h, 0, 0)
                    chain(wq_t, h, 0, 1)
                    chain(wq_t, h, 0, 2)
                    chain(wq_t, h, 0, 3)
                    chain(wq_t, h, 1, 1)
                    chain(wq_t, h, 1, 2)
                    chain(wq_t, h, 1, 3)
                    es_ax.close()
                    wp_pool = es_d.enter_context(
                        tc.tile_pool(name="wp_pool", bufs=1,
                                     side="right"))
                    ym_pool = es_d.enter_context(
                        tc.tile_pool(name="ym_pool", bufs=1,
                                     side="right"))
                    ot_pool = es_d.enter_context(
                        tc.tile_pool(name="ot_pool", bufs=1,
                                     side="right"))
                    dpools["wp"] = wp_pool
                    dpools["ym"] = ym_pool
                    dpools["ot"] = ot_pool
                    wp_t = wp_pool.tile([P, HPC, C], F16, tag="wp",
                                        name="wp")
                    dpools["wp_t"] = wp_t
                    for hb in range(4):
                        eng = nc.sync if hb % 2 == 0 else nc.scalar
                        eng.dma_start(
                            wp_t[:, 2 * hb:2 * hb + 2, :],
                            wp[:, 2 * hb:2 * hb + 2, :])
                    c_group(h, 0)
                    c_group(h, 1)

                    def chain_gens(*gens):
                        for g in gens:
                            yield from g

                    dpools["dgen"] = chain_gens(
                        d_tile_gen(0, psum_bc, lambda n: "psb", 2, True),
                        d_tile_gen(1, psum_bc, lambda n: "psb", 2, True),
                    )

                    def filler():
                        g = dpools.get("dgen")
                        if g is not None and \
                                next(g, StopIteration) is StopIteration:
                            dpools["dgen"] = None

                    c_group(h, 2, filler=filler)
                    c_group(h, 3, filler=filler)
            while pending:
                c_back()
            # finish any remainder of the interleaved m=0/1 tiles
            g = dpools.get("dgen")
            if g is not None:
                for _ in g:
                    pass
            es_bc.close()

            # ------------- phase D: projection (m=0,1 done above) ----
            with tc.tile_pool(name="psum_d", bufs=1,
                              space="PSUM") as psum_d:
                next_ym = 4
                for m in range(2, T // P):
                    while next_ym < min(T // P, m + 5):
                        load_ym(next_ym)
                        next_ym += 1
                    for _ in d_tile_gen(m, psum_d,
                                        lambda n: f"pso{n}", 2, False):
                        pass
            es_d.close()

    nc.finalize()
    return nc


def prep_inputs(x, w_attn, b_attn, w_proj, b_proj):
    """Build the 8 per-core input maps from full inputs.

    All tensors are repacked so SBUF partition rows are contiguous
    multi-KB runs in DRAM (fast DMA packets):
      xt  [P, TT, CC, NT]: xt[p,tt,c,n]  = x[tt*NT+n, c*P+p]
      wqk [P, HPC, CC, 2P]: per head-pair block, c-major
      wv  [P, CC, HPC*D]
      wp  [P, HPC, C]
    """
    x = np.asarray(x, dtype=np.float32)
    w_attn = np.asarray(w_attn, dtype=np.float32)
    b_attn = np.asarray(b_attn, dtype=np.float32)
    w_proj = np.asarray(w_proj, dtype=np.float32)

    scale = np.float32(1.0 / np.sqrt(D))

    inv_freq = 1.0 / (ROPE_BASE ** (np.arange(0, D, 2, dtype=np.float32) / D))
    tpos = np.arange(T, dtype=np.float32)
    ang = np.outer(tpos, inv_freq)  # [T, 64]
    cos_t, sin_t = np.cos(ang).T, np.sin(ang).T  # [64, T]
    cs = np.ascontiguousarray(
        np.concatenate([cos_t, cos_t], axis=0)).astype(np.float16)
    sw = np.ascontiguousarray(
        np.concatenate([-sin_t, sin_t], axis=0)).astype(np.float16)

    qq = np.arange(P)
    kk = np.arange(P)[:, None]
    tri = np.ascontiguousarray(
        (qq[None, :] >= kk).astype(np.float16))  # [128,128] causal triangle

    onesm = np.ones((P, P), dtype=np.float16)

    in_maps = []
    for core in range(8):
        b = core // 2
        hg = core % 2
        heads = list(range(hg * HPC, (hg + 1) * HPC))
        # interleaved feature order: (q_h, k_h) per head
        wqk_cols = []
        bqk_vals = []
        for h in heads:
            qcol = np.arange(h * D, (h + 1) * D)
            kcol = qcol + C
            wqk_cols.append(w_attn[:, qcol] * scale)
            wqk_cols.append(w_attn[:, kcol])
            bqk_vals.append(b_attn[qcol] * scale)
            bqk_vals.append(b_attn[kcol])
        wqk_full = np.concatenate(wqk_cols, axis=1)  # [C, NF*P]
        # -> [P, HPC, CC, 2P]
        wqk_s = np.ascontiguousarray(
            wqk_full.reshape(CC, P, HPC, 2 * P).transpose(1, 2, 0, 3)
        ).astype(np.float16)
        bqk_s = np.ascontiguousarray(
            np.stack(bqk_vals, axis=1)).astype(np.float32)  # [128, 16]

        vcols = np.concatenate(
            [np.arange(h * D, (h + 1) * D) for h in heads]) + 2 * C
        wv_full = w_attn[:, vcols]  # [C, HPC*D]
        wv_s = np.ascontiguousarray(
            wv_full.reshape(CC, P, HPC * D).transpose(1, 0, 2)
        ).astype(np.float16)
        pcols = np.concatenate(
            [np.arange(h * D, (h + 1) * D) for h in heads])
        wp_full = w_proj[pcols, :]  # [HPC*D, C]
        wp_s = np.ascontiguousarray(
            wp_full.reshape(HPC, P, C).transpose(1, 0, 2)
        ).astype(np.float16)
        # x: [T, C] -> [P, TT, CC, NT]
        xt_s = np.ascontiguousarray(
            x[b].T.reshape(CC, P, TT, NT).transpose(1, 2, 0, 3)
        ).astype(np.float16)

        in_maps.append({
            "xt": xt_s, "wqk": wqk_s, "bqk": bqk_s, "wv": wv_s,
            "cs": cs, "sw": sw, "tri": tri, "onesm": onesm, "wp": wp_s,
        })
    return in_maps


def _get_program():
    if "nc" not in _CACHE:
        _CACHE["nc"] = build_program()
    return _CACHE["nc"]


def _postprocess(outs, b_proj, bvp):
    # bvp[hg]: bv_core @ wp_core for head-group hg — the attention value
    # bias contributes a token-independent row to the projection output.
    base = np.asarray(b_proj, dtype=np.float32) + bvp[0] + bvp[1]
    return np.stack(
        [outs[2 * b].astype(np.float32) + outs[2 * b + 1].astype(np.float32)
         + base[None, :] for b in range(B)]
    ).astype(np.float32)


def _run(inputs, trace=False):
    from concourse.bass_utils import run_bass_kernel_spmd

    nc = _get_program()
    in_maps = prep_inputs(
        inputs["x"], inputs["w_attn"], inputs["b_attn"],
        inputs["w_proj"], inputs["b_proj"],
    )
    b_attn = np.asarray(inputs["b_attn"], dtype=np.float32)
    w_proj = np.asarray(inputs["w_proj"], dtype=np.float32)
    bvp = []
    for hg in range(2):
        cols = np.concatenate(
            [np.arange(h * D, (h + 1) * D)
             for h in range(hg * HPC, (hg + 1) * HPC)])
        bvp.append(b_attn[2 * C + cols] @ w_proj[cols, :])
    res = run_bass_kernel_spmd(nc, in_maps, core_ids=list(range(8)),
                               trace=trace)
    full = _postprocess([r["out"] for r in res.results],
                        inputs["b_proj"], bvp)
    return full, res


def kernel(**inputs):
    full, _ = _run(inputs, trace=False)
    return full


if __name__ == "__main__":
    _get_program()
    print("built ok")
